# revision 1
# baseline (speedup 1.0000x reference)
"""RBF Gram matrix kernel for Trainium2, 8-core SPMD.

K[i, j] = exp(-gamma * ||x_i - s_j||^2),  x [8192, 256] f32, support [8192, 256] f32.

Strategy:
  - Shard rows of x across 8 cores (1024 rows/core); replicate support.
  - exponent = (x.s - 0.5*||s||^2)/128 - ||x||^2/256.  The cross term runs as
    three fp8e4m3 DoubleRow matmuls (hi/lo splits, K=256 packed 2-per-cell,
    0.5 cyc/row), the s-norm as a fourth tiny DoubleRow matmul with hi/mid/lo
    fp8 rows, and the x-norm rides the ScalarE activation as a per-partition
    fp32 bias.  Epilogue: out = Exp(psum/128 + bias) written fp16 (halves the
    HBM store traffic vs f32), upcast to f32 on host.
  - Loop is support-column-chunk outer / x-row-tile inner so each loaded
    support chunk feeds 8 row-tiles of matmuls before the next chunk is
    needed; load DMAs stay ahead of the PE from the first group on.
"""

import numpy as np

try:
    import concourse.bass as bass  # noqa: F401
except ImportError:
    import sys

    sys.path.insert(0, "/opt/trn_rl_repo")

N, M, D = 8192, 8192, 256
GAMMA = 1.0 / D
NCORES = 8
STRIP = N // NCORES  # 1024 rows of x per core
P = 128
NTILE = 512  # matmul free-dim slice (one fp32 PSUM bank)
NGROUP = 2048  # activation/store group: 4 PSUM banks per ACTIVATE

_CACHE = {}


def _build(pe_warmup=300):
    import concourse.tile as tile
    from concourse import bacc, mybir

    f8 = mybir.dt.float8e4
    f16 = mybir.dt.float16
    f32 = mybir.dt.float32
    DR = mybir.MatmulPerfMode.DoubleRow

    nc = bacc.Bacc("TRN2", target_bir_lowering=False, debug=False, num_devices=NCORES)

    # Contraction element d = 128*i + k; hl = hi/lo split half.
    # Both operands are chunked on their second axis (m-tile for x, column
    # chunk for support) so every DMA write and every matmul read covers one
    # contiguous per-partition byte interval — the tile framework tracks
    # dependencies as bounding intervals, and precise intervals are what let
    # early tiles start as soon as their own chunk has landed.
    xx = nc.dram_tensor("xx", [P, STRIP // P, 2, 2, P], f8, kind="ExternalInput")
    sup = nc.dram_tensor("sup", [P, M // NTILE, 2, 2, NTILE], f8, kind="ExternalInput")
    fa = nc.dram_tensor("fa", [2, 2, M], f8, kind="ExternalInput")
    bv = nc.dram_tensor("bv", [P, STRIP // P], f32, kind="ExternalInput")
    out = nc.dram_tensor("out", [STRIP, M], f16, kind="ExternalOutput")

    n_mt = STRIP // P  # 8 m-tiles
    n_grp = M // NGROUP  # 4 support-column groups
    GW = NGROUP // NTILE  # 4 matmul slices per group

    with tile.TileContext(nc) as tc:
        with (
            tc.tile_pool(name="const", bufs=1) as const,
            tc.tile_pool(name="psum", bufs=2, space="PSUM") as psum_pool,
            tc.tile_pool(name="obuf", bufs=32) as obuf,
        ):
            xx_t = const.tile([P, STRIP // P, 2, 2, P], f8, tag="xx")
            sup_t = const.tile([P, M // NTILE, 2, 2, NTILE], f8, tag="sup")
            fa_t = const.tile([2, 2, M], f8, tag="fa")
            wa_t = const.tile([2, 2, P], f8, tag="wa")
            bv_t = const.tile([P, STRIP // P], f32, tag="bv")
            scr = const.tile([2, 2, 32], f8, tag="scr")

            # table-load bait: tiny Exp activation with no DMA deps so the
            # 1283ns activation-table load runs during the load phase
            nc.vector.memset(scr[:], 0)
            # aug weights: all-ones (the unused 4th virtual row multiplies
            # fa's zero row, so no zero mask is needed) -> on-core, no DMA
            nc.vector.memset(wa_t[:], 1.0)
            dummy = obuf.tile([2, 16], f16, tag="dummy")

            # loads: the (g=0, m=0) critical set first — the m=0 slice of x on
            # the SWDGE queue (earliest possible transfer), support chunks 0-3
            # interleaved across the two HWDGE queues
            nc.gpsimd.dma_start(out=xx_t[:, 0], in_=xx[:, 0])
            nc.gpsimd.dma_start(out=fa_t[:, :, :NGROUP], in_=fa[:, :, :NGROUP])
            nc.sync.dma_start(out=sup_t[:, 0], in_=sup[:, 0])
            nc.sync.dma_start(out=sup_t[:, 1], in_=sup[:, 1])
            nc.sync.dma_start(out=sup_t[:, 2], in_=sup[:, 2])
            nc.sync.dma_start(out=sup_t[:, 3], in_=sup[:, 3])
            nc.gpsimd.dma_start(out=bv_t[:], in_=bv[:])
            nc.gpsimd.dma_start(out=xx_t[:, 1], in_=xx[:, 1])
            nc.gpsimd.dma_start(out=xx_t[:, 2:], in_=xx[:, 2:])
            nc.gpsimd.dma_start(out=fa_t[:, :, NGROUP:], in_=fa[:, :, NGROUP:])
            for c in range(4, M // NTILE):
                nc.gpsimd.dma_start(out=sup_t[:, c], in_=sup[:, c])

            nc.scalar.activation(
                dummy[:], scr[:, 0, :].bitcast(f16), mybir.ActivationFunctionType.Exp
            )

            for g in range(n_grp):
                for m in range(n_mt):
                    ms = slice(m * P, (m + 1) * P)
                    ps = psum_pool.tile([P, NGROUP], f32)
                    first = g == 0 and m == 0
                    if pe_warmup and first:
                        # keep the PE continuously busy (nearly free per the
                        # cost model) through the load phase so the p-state
                        # ramp completes before the real matmuls start
                        for _ in range(pe_warmup):
                            nc.tensor.matmul(
                                ps[:32, :8], scr[:], scr[:, :, :8],
                                start=True, stop=True,
                                perf_mode=DR, skip_group_check=True,
                            )

                    def mm(c, k, flags=None):
                        if c == 3:
                            lhsT, rhs = wa_t[:], None
                        else:
                            # (xh,sh), (xl,sh), (xh,sl)
                            xi, hl = ((0, 0), (1, 0), (0, 1))[c]
                            lhsT = xx_t[:, m, :, xi, :]
                        n0 = g * NGROUP + k * NTILE
                        rhs = (
                            fa_t[:, :, n0 : n0 + NTILE]
                            if c == 3
                            else sup_t[:, g * GW + k, :, ((0, 0), (1, 0), (0, 1))[c][1], :]
                        )
                        st, sp = flags if flags else (c == 0, c == 3)
                        nc.tensor.matmul(
                            ps[:, k * NTILE : (k + 1) * NTILE],
                            lhsT,
                            rhs,
                            start=st,
                            stop=sp,
                            perf_mode=DR,
                        )

                    if first:
                        # banks 0-2: cross passes then their augs (fa lands
                        # mid-stream); bank 3 runs aug-FIRST (start=True) so
                        # only its 3 cross MMs trail the last support chunk
                        for k in range(GW - 1):
                            for c in range(3):
                                mm(c, k)
                        for k in range(GW - 1):
                            mm(3, k)
                        mm(3, GW - 1, flags=(True, False))
                        mm(0, GW - 1, flags=(False, False))
                        mm(1, GW - 1, flags=(False, False))
                        mm(2, GW - 1, flags=(False, True))
                    else:
                        for c in range(4):
                            for k in range(GW):
                                mm(c, k)
                    ot = obuf.tile([P, NGROUP], f16)
                    last = g == n_grp - 1 and m == n_mt - 1
                    # last group: asymmetric act/store split so the final
                    # store (the drain-tail anchor) is small
                    bounds = [0, 1280, NGROUP] if last else [0, NGROUP]
                    for j in range(len(bounds) - 1):
                        js = slice(bounds[j], bounds[j + 1])
                        nc.scalar.activation(
                            ot[:, js],
                            ps[:, js],
                            mybir.ActivationFunctionType.Exp,
                            bias=bv_t[:, m : m + 1],
                            scale=2.0 * GAMMA,
                        )
                        gs = slice(g * NGROUP + js.start, g * NGROUP + js.stop)
                        nc.sync.dma_start(out=out[ms, gs], in_=ot[:, js])
    nc.compile()
    return nc


def _pack(v):
    """[R, 256] fp8-values -> [128, 2, R] with [k, i, r] = v[r, 128*i + k]."""
    return np.ascontiguousarray(v.T.reshape(2, P, -1).transpose(1, 0, 2))


def kernel(x, support):
    import ml_dtypes

    from concourse.bass_utils import run_bass_kernel_spmd

    if "nc" not in _CACHE:
        _CACHE["nc"] = _build()
    nc = _CACHE["nc"]

    f8 = ml_dtypes.float8_e4m3

    x = np.asarray(x, dtype=np.float32)
    support = np.asarray(support, dtype=np.float32)

    def split8(v):
        hi = v.astype(f8)
        lo = (v - hi.astype(np.float32)).astype(f8)
        return hi, lo

    xhi, xlo = split8(x)
    shi, slo = split8(support)

    x_sq = np.einsum("nd,nd->n", x, x)
    s_sq = np.einsum("md,md->m", support, support)

    # aug moving rows: -0.5*s_sq as fp8 hi/mid/lo (residual after 3 casts
    # is ~6e-3 absolute -> ~5e-5 relative on the output exponent)
    v = -0.5 * s_sq
    sqh = v.astype(f8)
    r = v - sqh.astype(np.float32)
    sqm = r.astype(f8)
    sql = (r - sqm.astype(np.float32)).astype(f8)
    fa = np.zeros((2, 2, M), f8)
    fa[0, 0], fa[0, 1], fa[1, 0] = sqh, sqm, sql
    # [k, chunk, i, hl, 512]
    sup_full = np.ascontiguousarray(
        np.stack([_pack(shi), _pack(slo)], axis=2)
        .reshape(P, 2, 2, M // NTILE, NTILE)
        .transpose(0, 3, 1, 2, 4)
    )
    xx_full = np.ascontiguousarray(np.stack([_pack(xhi), _pack(xlo)], axis=2))
    bv_full = (-x_sq / 256.0).astype(np.float32)

    in_maps = []
    for c in range(NCORES):
        cs = slice(c * STRIP, (c + 1) * STRIP)
        in_maps.append(
            {
                "xx": np.ascontiguousarray(
                    xx_full[:, :, :, cs]
                    .reshape(P, 2, 2, STRIP // P, P)
                    .transpose(0, 3, 1, 2, 4)
                ),
                "sup": sup_full,
                "fa": fa,
                "bv": np.ascontiguousarray(
                    bv_full[cs].reshape(STRIP // P, P).T
                ),
            }
        )

    res = run_bass_kernel_spmd(nc, in_maps, list(range(NCORES)))
    return np.concatenate(
        [res.results[c]["out"].astype(np.float32) for c in range(NCORES)], axis=0
    )

